# revision 1
# baseline (speedup 1.0000x reference)
"""GAT layer (AdaptiveBreadthLayer) on 8 TRN2 NeuronCores.

Strategy:
  - dst-shard: core c owns destination nodes [c*6272, (c+1)*6272) (N padded
    50000 -> 50176). Every edge lives on exactly one core (by dst), so no
    cross-core reduction and no collectives are needed.
  - Each core redundantly computes the full projection table
    row(n) = [feat(n) (256) | el(n) (4) | er(n) (4) | pad] in bf16
    (phase 1), stored in its local DRAM, split into two halves so rows are
    indexable with int16 for dma_gather.
  - Phase 2 walks the core's destination tiles (128 dst nodes each, load
    balanced by in-degree binning). Per tile: dma_gather of table rows for
    the tile's edges' sources, one-hot (edge -> dst-slot) matmuls for the
    segment softmax denominator and the weighted feature aggregation.
    Per-dst er values ride along as one reserved "pseudo-edge" chunk per
    table half (blended by a per-core lo/hi mask). Softmax runs without
    max-subtraction (values are small, mathematically identical) and the
    1/denom normalization is applied per destination after aggregation.
"""

import sys

import numpy as np

sys.path.insert(0, "/opt/trn_rl_repo")

import ml_dtypes

import concourse.bacc as bacc
import concourse.bass as bass
import concourse.mybir as mybir
from concourse.tile import TileContext

BF16 = mybir.dt.bfloat16
F32 = mybir.dt.float32
I32 = mybir.dt.int32
I16 = mybir.dt.int16

P = 128
H = 4
D = 64
HD = H * D  # 256
ROWP = 384  # padded table row: feat(256) | el(4) | er(4) | pad -> 768B
IN_DIM = 256
NEG_SLOPE = 0.2

N = 50000
E = 800000
NC = 8
N_PAD = 50176  # 8 * 49 * 128
NR = N_PAD // NC  # 6272 rows per core
TILES = NR // P  # 49 dst tiles per core
HALF = N_PAD // 2  # 25088 rows per table half (int16-indexable)
PAD_DSTLOC = 200.0  # any value >= 128: matches no dst slot
MAXC = 8  # dma_gather ucode caps at 1024 indices per instruction
SIM_INIT = False  # init padding for the interpreter's uninit-memory checks


# --------------------------------------------------------------------------
# host-side preprocessing (index structures only; no float math)
# --------------------------------------------------------------------------

def _prep_core(src_c, dst_c, base):
    """Bin a core's dst nodes into TILES bins of P nodes balanced by
    in-degree."""
    dst_local = dst_c - base
    indeg = np.bincount(dst_local, minlength=NR)
    order = np.argsort(-indeg, kind="stable")  # desc by degree
    rounds = order.reshape(P, TILES).copy()  # snake-fill P rounds x TILES bins
    rounds[1::2] = rounds[1::2, ::-1]
    members = rounds
    tile_of = np.empty(NR, dtype=np.int64)
    pos_of = np.empty(NR, dtype=np.int64)
    tile_of[members.ravel()] = np.tile(np.arange(TILES), P)
    pos_of[members.ravel()] = np.repeat(np.arange(P), TILES)

    counts = indeg[members].sum(axis=0)
    tile_order = np.argsort(-counts, kind="stable")
    rank_of_tile = np.empty(TILES, dtype=np.int64)
    rank_of_tile[tile_order] = np.arange(TILES)

    member_ids = members[:, tile_order] + base  # [P, TILES] global ids
    t_e = rank_of_tile[tile_of[dst_local]]
    p_e = pos_of[dst_local]
    return member_ids, t_e, p_e


def preprocess(src, dst):
    src = np.asarray(src).astype(np.int64)
    dst = np.asarray(dst).astype(np.int64)
    bf = ml_dtypes.bfloat16
    core_of = dst // NR
    per_core = []
    lo_counts = np.zeros((NC, TILES), dtype=np.int64)
    hi_counts = np.zeros((NC, TILES), dtype=np.int64)
    for c in range(NC):
        m = core_of == c
        member_ids, t_e, p_e = _prep_core(src[m], dst[m], c * NR)
        is_lo = src[m] < HALF
        per_core.append((src[m], member_ids, t_e, p_e, is_lo))
        np.add.at(lo_counts[c], t_e[is_lo], 1)
        np.add.at(hi_counts[c], t_e[~is_lo], 1)
    # +1: chunk 0 of each half block is reserved for member pseudo-edges
    clo = np.ceil(lo_counts.max(axis=0) / P).astype(np.int64) + 1
    chi = np.ceil(hi_counts.max(axis=0) / P).astype(np.int64) + 1
    c_tot = clo + chi
    # per-tile aux width (int16 cols): idx C*8 | dstloc C | m_lo, m_hi
    widths = c_tot * 9 + 2
    aux_offs = np.concatenate([[0], np.cumsum(widths)[:-1]])
    sum_w = int(widths.sum())
    chunk_offs = np.concatenate([[0], np.cumsum(c_tot)[:-1]])

    aux = []
    for c in range(NC):
        src_c, member_ids, t_e, p_e, is_lo = per_core[c]
        auxw = np.zeros((P, sum_w), dtype=np.int16)
        dl = np.full((P, int(c_tot.sum())), PAD_DSTLOC, dtype=bf)
        for half, res_off in ((True, 0), (False, None)):
            sel = is_lo == half
            t_h = t_e[sel]
            s_h = src_c[sel] - (0 if half else HALF)
            p_h = p_e[sel]
            order = np.argsort(t_h, kind="stable")
            t_s, s_s, p_s = t_h[order], s_h[order], p_h[order]
            tile_starts = np.searchsorted(t_s, np.arange(TILES))
            q = np.arange(len(order)) - tile_starts[t_s]
            # lo block: chunk 0 reserved, real 1..clo-1
            # hi block: real clo..C-2, chunk C-1 reserved
            local_chunk = (1 if half else clo[t_s]) + q // P
            chunk = chunk_offs[t_s] + local_chunk
            slot = q % P
            dl[slot, chunk] = p_s.astype(bf)
            icol = aux_offs[t_s] + local_chunk * 8 + slot // 16
            irow = slot % 16
            auxw[irow, icol] = s_s.astype(np.int16)
        # member pseudo-edges into the reserved chunks (own half only)
        my_lo = c < NC // 2
        for t in range(TILES):
            mem = member_ids[:, t] - (0 if my_lo else HALF)
            rchunk = 0 if my_lo else (clo[t] + chi[t] - 1)
            icol = aux_offs[t] + rchunk * 8 + np.arange(P) // 16
            auxw[np.arange(P) % 16, icol] = mem.astype(np.int16)
        # idx regions are wrapped in 16 partitions; replicate to all 8 groups
        for t in range(TILES):
            sl = slice(int(aux_offs[t]), int(aux_offs[t] + c_tot[t] * 8))
            auxw[:, sl] = np.tile(auxw[0:16, sl], (8, 1))
        # dstloc + masks into aux
        for t in range(TILES):
            c0, ct = chunk_offs[t], c_tot[t]
            base = aux_offs[t] + ct * 8
            auxw[:, base : base + ct] = dl[:, c0 : c0 + ct].view(np.int16)
            m_lo = np.float32(1.0 if my_lo else 0.0)
            auxw[:, base + ct] = np.full(P, m_lo, dtype=bf).view(np.int16)
            auxw[:, base + ct + 1] = np.full(P, 1.0 - m_lo, dtype=bf).view(np.int16)
        aux.append(
            dict(
                auxw=auxw,
                member_ids=np.ascontiguousarray(member_ids.astype(np.int32)),
            )
        )
    return aux, [int(x) for x in clo], [int(x) for x in chi]


# --------------------------------------------------------------------------
# device kernel builder
# --------------------------------------------------------------------------

def build_kernel(n_pad, tiles, clo, chi):
    c_tot = [a + b for a, b in zip(clo, chi)]
    widths = [ct * 9 + 2 for ct in c_tot]
    sum_w = int(sum(widths))
    half = n_pad // 2
    nc = bacc.Bacc()

    hT = nc.declare_dram_parameter("hT", [IN_DIM, n_pad], BF16, isOutput=False)
    Wb = nc.declare_dram_parameter("Wb", [IN_DIM, HD], BF16, isOutput=False)
    WTb = nc.declare_dram_parameter("WTb", [IN_DIM, HD], BF16, isOutput=False)
    ALR = nc.declare_dram_parameter("ALR", [IN_DIM, 2 * H], BF16, isOutput=False)
    bias_rep = nc.declare_dram_parameter("bias_rep", [P, HD], F32, isOutput=False)
    cmax = max(c_tot)
    iota_big = nc.declare_dram_parameter(
        "iota_big", [P, P * cmax], BF16, isOutput=False
    )
    ident = nc.declare_dram_parameter("ident", [P, P], BF16, isOutput=False)
    auxw = nc.declare_dram_parameter("auxw", [P, sum_w], I16, isOutput=False)
    out = nc.declare_dram_parameter("out", [tiles * P, D], F32, isOutput=True)

    AL = mybir.AluOpType
    KCH = IN_DIM // P  # 2 contraction chunks

    with TileContext(nc) as tc:
        with (
            tc.tile_pool(name="const", bufs=1) as constp,
            tc.tile_pool(name="dram", bufs=1, space="DRAM") as dramp,
        ):
            t_lo = dramp.tile([half, ROWP], BF16)
            t_hi = dramp.tile([half, ROWP], BF16)

            W_sb = constp.tile([P, KCH * HD], BF16)
            WT_sb = constp.tile([P, KCH * HD], BF16)
            ALR_sb = constp.tile([P, KCH * 2 * H], BF16)
            WALR_sb = constp.tile([P, KCH * 2 * H], BF16)
            bias_sb = constp.tile([P, HD], F32)
            iota_sb = constp.tile([P, P * cmax], BF16)
            ident_sb = constp.tile([P, P], BF16)
            for kk in range(KCH):
                nc.sync.dma_start(
                    out=W_sb[:, kk * HD : (kk + 1) * HD],
                    in_=Wb[kk * P : (kk + 1) * P, :],
                )
                nc.sync.dma_start(
                    out=WT_sb[:, kk * HD : (kk + 1) * HD],
                    in_=WTb[kk * P : (kk + 1) * P, :],
                )
                nc.sync.dma_start(
                    out=ALR_sb[:, kk * 2 * H : (kk + 1) * 2 * H],
                    in_=ALR[kk * P : (kk + 1) * P, :],
                )
            nc.sync.dma_start(out=bias_sb[:], in_=bias_rep[:, :])
            nc.sync.dma_start(out=iota_sb[:], in_=iota_big[:, :])
            nc.sync.dma_start(out=ident_sb[:], in_=ident[:, :])

            # WALR = W @ ALR
            with tc.tile_pool(name="setup_ps", bufs=1, space="PSUM") as setupps:
                for ic in range(KCH):
                    walr_ps = setupps.tile([P, 2 * H], F32)
                    for kk in range(KCH):
                        nc.tensor.matmul(
                            walr_ps[:],
                            lhsT=WT_sb[:, kk * HD + ic * P : kk * HD + (ic + 1) * P],
                            rhs=ALR_sb[:, kk * 2 * H : (kk + 1) * 2 * H],
                            start=(kk == 0),
                            stop=(kk == KCH - 1),
                        )
                    nc.vector.tensor_copy(
                        out=WALR_sb[:, ic * 2 * H : (ic + 1) * 2 * H], in_=walr_ps[:]
                    )

            # ------------------- phase 1: projection table -------------------
            OB = 1024  # rows per outer block
            SUBS = OB // P
            n_ob = n_pad // OB
            with (
                tc.tile_pool(name="p1", bufs=3) as p1,
                tc.tile_pool(name="p1ps", bufs=3, space="PSUM") as p1ps,
            ):
                for ob in range(n_ob):
                    start = ob * OB
                    hT_t = p1.tile([P, KCH, OB], BF16, name="hT_t", tag="hT_t")
                    for kk in range(KCH):
                        nc.sync.dma_start(
                            out=hT_t[:, kk, :],
                            in_=hT[kk * P : (kk + 1) * P, start : start + OB],
                        )
                    stage = p1.tile([P, SUBS, ROWP], BF16, name="stage", tag="stage")
                    if SIM_INIT:
                        # pad cols never read downstream; zero for the sim
                        nc.vector.memset(stage[:, :, HD + 2 * H : ROWP], 0.0)
                    for sub in range(SUBS):
                        feat_ps = p1ps.tile([P, HD], F32, name="feat_ps", tag="feat_ps")
                        elr_ps = p1ps.tile([P, 2 * H], F32, name="elr_ps", tag="elr_ps")
                        for kk in range(KCH):
                            lh = hT_t[:, kk, sub * P : (sub + 1) * P]
                            nc.tensor.matmul(
                                feat_ps[:],
                                lhsT=lh,
                                rhs=W_sb[:, kk * HD : (kk + 1) * HD],
                                start=(kk == 0),
                                stop=(kk == KCH - 1),
                            )
                            nc.tensor.matmul(
                                elr_ps[:],
                                lhsT=lh,
                                rhs=WALR_sb[:, kk * 2 * H : (kk + 1) * 2 * H],
                                start=(kk == 0),
                                stop=(kk == KCH - 1),
                            )
                        nc.any.tensor_copy(out=stage[:, sub, 0:HD], in_=feat_ps[:])
                        nc.any.tensor_copy(
                            out=stage[:, sub, HD : HD + 2 * H], in_=elr_ps[:]
                        )
                    # one batched write per block (split if straddling halves)
                    ranges = []
                    if start + OB <= half:
                        ranges.append((0, SUBS, t_lo, start))
                    elif start >= half:
                        ranges.append((0, SUBS, t_hi, start - half))
                    else:
                        sub_split = (half - start) // P
                        ranges.append((0, sub_split, t_lo, start))
                        ranges.append((sub_split, SUBS, t_hi, 0))
                    wcols = ROWP if SIM_INIT else HD + 2 * H
                    for s0, s1, tgt, r0 in ranges:
                        nsub = s1 - s0
                        dst_ap = tgt[r0 : r0 + nsub * P, 0:wcols].rearrange(
                            "(s p) c -> p s c", p=P
                        )
                        nc.sync.dma_start(out=dst_ap, in_=stage[:, s0:s1, 0:wcols])

            # ------------------- phase 2: edge aggregation -------------------
            with (
                tc.tile_pool(name="p2", bufs=3) as p2,
                tc.tile_pool(name="p2s", bufs=4) as p2s,
                tc.tile_pool(name="outps", bufs=2, space="PSUM") as outps_pool,
                tc.tile_pool(name="ergps", bufs=2, space="PSUM") as ergps_pool,
                tc.tile_pool(name="sps", bufs=4, space="PSUM") as sps_pool,
            ):
                aux_off = 0
                for t in range(tiles):
                    CL, CH = int(clo[t]), int(chi[t])
                    C = CL + CH
                    W_t = C * 9 + 2
                    aux_t = p2.tile([P, W_t], I16, name="aux_t", tag="aux")
                    nc.sync.dma_start(
                        out=aux_t[:], in_=auxw[:, aux_off : aux_off + W_t]
                    )
                    idx_v = aux_t[:, 0 : C * 8]
                    dl_v = aux_t[:, C * 8 : C * 9].bitcast(BF16)
                    mlo_v = aux_t[:, C * 9 : C * 9 + 1].bitcast(BF16)
                    mhi_v = aux_t[:, C * 9 + 1 : C * 9 + 2].bitcast(BF16)

                    G = p2.tile([P, C * ROWP], BF16, name="G", tag="G")
                    for base, width, tb in ((0, CL, t_lo), (CL, CH, t_hi)):
                        done = 0
                        while done < width:
                            w = min(MAXC, width - done)
                            b = base + done
                            nc.gpsimd.dma_gather(
                                out_ap=G[:, b * ROWP : (b + w) * ROWP].rearrange(
                                    "p (c r) -> p c r", c=w
                                ),
                                in_ap=tb[:, :],
                                idxs_ap=idx_v[:, b * 8 : (b + w) * 8],
                                num_idxs=w * P,
                                num_idxs_reg=w * P,
                                elem_size=ROWP,
                            )
                            done += w

                    # blend member er rows from the two reserved chunks
                    # (chunk 0 of the lo block, chunk C-1 of the hi block)
                    erA = G[:, 260:264]
                    erB = G[:, (C - 1) * ROWP + 260 : (C - 1) * ROWP + 264]
                    e1 = p2s.tile([P, H], BF16, name="e1", tag="e1")
                    nc.vector.tensor_tensor(
                        out=e1[:], in0=erA, in1=mlo_v.to_broadcast([P, H]), op=AL.mult
                    )
                    er_t = p2s.tile([P, H], BF16, name="er_t", tag="er")
                    nc.vector.scalar_tensor_tensor(
                        out=er_t[:],
                        in0=erB,
                        scalar=mhi_v[:, 0:1],
                        in1=e1[:],
                        op0=AL.mult,
                        op1=AL.add,
                    )

                    # one-hot, chunk-innermost for the DVE 2x mode:
                    # ST3[e, d, j] = (dstloc[e, j] == d)
                    NRJ = C - 2  # real chunks are 1..C-2 (contiguous)
                    ST3 = p2.tile([P, P * C], BF16, name="ST3", tag="ST")
                    nc.vector.tensor_tensor(
                        out=ST3[:].rearrange("p (d c) -> p d c", d=P),
                        in0=dl_v.rearrange("p (one c) -> p one c", one=1).to_broadcast(
                            [P, P, C]
                        ),
                        in1=iota_sb[:]
                        .rearrange("p (d c) -> p d c", d=P)[:, :, 0:C],
                        op=AL.is_equal,
                    )
                    st_j = lambda j: ST3[:].rearrange("p (d c) -> p d c", d=P)[:, :, j]

                    # er gathered per edge: erg[e, h] = sum_d S[d, e] er_t[d, h]
                    erg_ps = ergps_pool.tile([P, C * H], F32, name="erg_ps")
                    for j in range(1, C - 1):
                        s_ps = sps_pool.tile([P, P], BF16, name="s_ps", tag="s_ps")
                        nc.tensor.transpose(
                            out=s_ps[:], in_=st_j(j), identity=ident_sb[:]
                        )
                        s_sb = p2s.tile([P, P], BF16, name="s_sb", tag="s_sb")
                        nc.any.tensor_copy(out=s_sb[:], in_=s_ps[:])
                        nc.tensor.matmul(
                            erg_ps[:, j * H : (j + 1) * H],
                            lhsT=s_sb[:],
                            rhs=er_t[:],
                            start=True,
                            stop=True,
                        )

                    # e_val = leaky_relu(el[src] + er[dst]); ex = exp(e_val)
                    ev = p2.tile([P, NRJ * H], F32, name="ev", tag="ev")
                    nc.vector.tensor_tensor(
                        out=ev[:].rearrange("p (c h) -> p c h", c=NRJ),
                        in0=G[:].rearrange("p (c r) -> p c r", c=C)[
                            :, 1 : C - 1, HD : HD + H
                        ],
                        in1=erg_ps[:].rearrange("p (c h) -> p c h", c=C)[:, 1 : C - 1, :],
                        op=AL.add,
                    )
                    lrel = p2.tile([P, NRJ * H], F32, name="lrel", tag="lrel")
                    nc.vector.scalar_tensor_tensor(
                        out=lrel[:],
                        in0=ev[:],
                        scalar=NEG_SLOPE,
                        in1=ev[:],
                        op0=AL.mult,
                        op1=AL.max,
                    )
                    # gx chunk layout: [ gs (256) | ex (4) ] so one matmul does
                    # both the weighted scatter and the softmax denominator
                    GX = HD + H  # 260
                    gx = p2.tile([P, C * GX], BF16, name="gx", tag="gx")
                    exb = p2.tile([P, NRJ * H], BF16, name="exb", tag="exb")
                    nc.scalar.activation(
                        out=exb[:], in_=lrel[:], func=mybir.ActivationFunctionType.Exp
                    )
                    nc.vector.tensor_copy(
                        out=gx[:].rearrange("p (c g) -> p c g", c=C)[
                            :, 1 : C - 1, HD : HD + H
                        ],
                        in_=exb[:],
                    )
                    nc.vector.tensor_tensor(
                        out=gx[:]
                        .rearrange("p (c g) -> p c g", c=C)[:, 1 : C - 1, 0:HD]
                        .rearrange("p c (h d) -> p c h d", h=H),
                        in0=G[:]
                        .rearrange("p (c r) -> p c r", c=C)[:, 1 : C - 1, 0:HD]
                        .rearrange("p c (h d) -> p c h d", h=H),
                        in1=exb[:]
                        .rearrange("p (c h one) -> p c h one", h=H, one=1)
                        .to_broadcast([P, NRJ, H, D]),
                        op=AL.mult,
                    )

                    out_ps = outps_pool.tile([P, GX], F32, name="out_ps")
                    for jj, j in enumerate(range(1, C - 1)):
                        nc.tensor.matmul(
                            out_ps[:],
                            lhsT=st_j(j),
                            rhs=gx[:, j * GX : (j + 1) * GX],
                            start=(jj == 0),
                            stop=(jj == NRJ - 1),
                        )

                    # epilogue: normalize, bias, tanh, mean over heads
                    rd0 = p2s.tile([P, H], F32, name="rd0", tag="rd0")
                    nc.vector.tensor_scalar(
                        out=rd0[:],
                        in0=out_ps[:, HD : HD + H],
                        scalar1=1e-9,
                        scalar2=None,
                        op0=AL.max,
                    )
                    rd = p2s.tile([P, H], F32, name="rd", tag="rd")
                    nc.vector.reciprocal(out=rd[:], in_=rd0[:])
                    nrm = p2.tile([P, HD], F32, name="nrm", tag="nrm")
                    nc.vector.tensor_tensor(
                        out=nrm[:].rearrange("p (h d) -> p h d", h=H),
                        in0=out_ps[:, 0:HD].rearrange("p (h d) -> p h d", h=H),
                        in1=rd[:]
                        .rearrange("p (h one) -> p h one", one=1)
                        .to_broadcast([P, H, D]),
                        op=AL.mult,
                    )
                    nb = p2.tile([P, HD], F32, name="nb", tag="nb")
                    nc.gpsimd.tensor_tensor(
                        out=nb[:], in0=nrm[:], in1=bias_sb[:], op=AL.add
                    )
                    th = p2.tile([P, HD], F32, name="th", tag="th")
                    nc.scalar.activation(
                        out=th[:], in_=nb[:], func=mybir.ActivationFunctionType.Tanh
                    )
                    m1 = p2s.tile([P, D], F32, name="m1", tag="m1")
                    nc.gpsimd.tensor_tensor(
                        out=m1[:], in0=th[:, 0:D], in1=th[:, D : 2 * D], op=AL.add
                    )
                    m2 = p2s.tile([P, D], F32, name="m2", tag="m2")
                    nc.gpsimd.tensor_tensor(
                        out=m2[:],
                        in0=th[:, 2 * D : 3 * D],
                        in1=th[:, 3 * D : 4 * D],
                        op=AL.add,
                    )
                    m3 = p2s.tile([P, D], F32, name="m3", tag="m3")
                    nc.gpsimd.tensor_tensor(out=m3[:], in0=m1[:], in1=m2[:], op=AL.add)
                    of = p2.tile([P, D], F32, name="of", tag="of")
                    nc.gpsimd.tensor_scalar(
                        out=of[:], in0=m3[:], scalar1=0.25, scalar2=None, op0=AL.mult
                    )
                    nc.sync.dma_start(out=out[t * P : (t + 1) * P, :], in_=of[:])
                    aux_off += W_t
    return nc


# --------------------------------------------------------------------------
# host entry
# --------------------------------------------------------------------------

def _make_static_inputs(h, W, attn_l, attn_r, bias):
    bf = ml_dtypes.bfloat16
    h_pad = np.zeros((N_PAD, IN_DIM), dtype=np.float32)
    h_pad[:N] = np.asarray(h, dtype=np.float32)
    hT = np.ascontiguousarray(h_pad.T).astype(bf)
    Wb = np.asarray(W, dtype=np.float32).astype(bf)
    WTb = np.ascontiguousarray(np.asarray(W, dtype=np.float32).T).astype(bf)
    ALRm = np.zeros((IN_DIM, 2 * H), dtype=np.float32)
    al = np.asarray(attn_l, dtype=np.float32)
    ar = np.asarray(attn_r, dtype=np.float32)
    for hh in range(H):
        ALRm[hh * D : (hh + 1) * D, hh] = al[hh]
        ALRm[hh * D : (hh + 1) * D, H + hh] = ar[hh]
    ALRm = ALRm.astype(bf)
    bias_rep = np.tile(np.asarray(bias, dtype=np.float32).reshape(1, HD), (P, 1))
    ident = np.eye(P, dtype=np.float32).astype(bf)
    return dict(
        hT=hT,
        Wb=Wb,
        WTb=WTb,
        ALR=ALRm,
        bias_rep=np.ascontiguousarray(bias_rep),
        ident=np.ascontiguousarray(ident),
    )


def make_iota_big(cmax):
    # iota_big[p, d*cmax + j] = d  (chunk-innermost iota for the ST3 build)
    row = np.repeat(np.arange(P, dtype=np.float32), cmax).reshape(1, P * cmax)
    return np.ascontiguousarray(
        np.tile(row, (P, 1)).astype(ml_dtypes.bfloat16)
    )


def bench(nc, in_maps, n_iters=10):
    """Repeated-execution wall timing of the compiled SPMD kernel via PJRT.

    Returns (per_call_seconds_list, results_of_last_call)."""
    import time

    import jax
    from jax.sharding import Mesh, NamedSharding, PartitionSpec
    from jax.experimental.shard_map import shard_map

    from concourse import bass2jax, mybir as _mb

    bass2jax.install_neuronx_cc_hook()
    n_cores = len(in_maps)
    in_names, out_names, out_avals, zero_outs = [], [], [], []
    partition_name = nc.partition_id_tensor.name if nc.partition_id_tensor else None
    for alloc in nc.m.functions[0].allocations:
        if not isinstance(alloc, _mb.MemoryLocationSet):
            continue
        name = alloc.memorylocations[0].name
        if alloc.kind == "ExternalInput":
            if name != partition_name:
                in_names.append(name)
        elif alloc.kind == "ExternalOutput":
            out_names.append(name)
            shape = tuple(alloc.tensor_shape)
            dtype = _mb.dt.np(alloc.dtype)
            out_avals.append(jax.core.ShapedArray(shape, dtype))
            zero_outs.append(np.zeros(shape, dtype))
    n_params = len(in_names)
    all_in_names = in_names + out_names
    if partition_name is not None:
        all_in_names.append(partition_name)

    def _body(*args):
        operands = list(args)
        if partition_name is not None:
            operands.append(bass2jax.partition_id_tensor())
        outs = bass2jax._bass_exec_p.bind(
            *operands,
            out_avals=tuple(out_avals),
            in_names=tuple(all_in_names),
            out_names=tuple(out_names),
            lowering_input_output_aliases=(),
            sim_require_finite=True,
            sim_require_nnan=True,
            nc=nc,
        )
        return tuple(outs)

    devices = jax.devices()[:n_cores]
    mesh = Mesh(np.asarray(devices), ("core",))
    n_outs = len(out_names)
    sharded = jax.jit(
        shard_map(
            _body,
            mesh=mesh,
            in_specs=(PartitionSpec("core"),) * (n_params + n_outs),
            out_specs=(PartitionSpec("core"),) * n_outs,
            check_rep=False,
        ),
        keep_unused=True,
    )
    sh = NamedSharding(mesh, PartitionSpec("core"))
    concat_in = [
        jax.device_put(
            np.concatenate([np.asarray(in_maps[c][nm]) for c in range(n_cores)], 0), sh
        )
        for nm in in_names
    ]
    concat_zeros = [
        jax.device_put(np.zeros((n_cores * z.shape[0], *z.shape[1:]), z.dtype), sh)
        for z in zero_outs
    ]
    outs = sharded(*concat_in, *concat_zeros)  # warmup/compile
    jax.block_until_ready(outs)
    times = []
    for _ in range(n_iters):
        t0 = time.perf_counter()
        outs = sharded(*concat_in, *concat_zeros)
        jax.block_until_ready(outs)
        times.append(time.perf_counter() - t0)
    results = [
        {
            nm: np.asarray(outs[i]).reshape(n_cores, *out_avals[i].shape)[c]
            for i, nm in enumerate(out_names)
        }
        for c in range(n_cores)
    ]
    return times, results


def kernel(h, W, attn_l, attn_r, bias, src, dst):
    from concourse.bass_utils import run_bass_kernel_spmd

    aux, clo, chi = preprocess(src, dst)
    static = _make_static_inputs(h, W, attn_l, attn_r, bias)
    nc = build_kernel(N_PAD, TILES, clo, chi)
    nc.compile()  # bacc passes: matmul wait splitting, event sems, DCE
    iota_big = make_iota_big(max(a + b for a, b in zip(clo, chi)))
    in_maps = []
    for c in range(NC):
        m = dict(static)
        m["auxw"] = aux[c]["auxw"]
        m["iota_big"] = iota_big
        in_maps.append(m)
    res = run_bass_kernel_spmd(nc, in_maps, core_ids=list(range(NC)), trace=False)
    out_full = np.zeros((N, D), dtype=np.float32)
    for c in range(NC):
        dev = res.results[c]["out"]  # [TILES*P, D]
        ids = aux[c]["member_ids"]  # [P, TILES]
        rows = ids.T.reshape(-1)  # row t*P+p  <->  ids[p, t]
        valid = rows < N
        out_full[rows[valid]] = dev[valid]
    kernel.last_nc = nc
    kernel.last_in_maps = in_maps
    kernel.last_aux = aux
    return out_full



# revision 5
# speedup vs baseline: 1.1113x; 1.1113x over previous
"""GAT layer (AdaptiveBreadthLayer) on 8 TRN2 NeuronCores.

Strategy (v2):
  - dst-shard: core c owns destination nodes [c*6272, (c+1)*6272) (N padded
    50000 -> 50176). Every edge lives on exactly one core (by dst), so no
    cross-core reduction and no collectives are needed.
  - Each core redundantly computes a full projection table with PER-CORE
    PERMUTED row order (its own member nodes first, in (tile,pos) order) so
    member er/el values are readable with a tiny contiguous DMA at uniform
    SPMD addresses. Rows are 512B (the DMA-gather sweet spot):
      {el 4xbf16 | er 4xbf16 | feat dims d<60 (d,h)-major bf16 (480B)
       | feat dims d>=60 (d,h)-major fp8e4m3 (16B)}
    The (d,h)-major layout makes the per-edge exp-weighting multiply a
    packed-bf16 DVE op (2x mode) with the broadcast on a middle dim.
  - Phase 2 walks the core's 49 destination tiles (128 dst nodes each,
    degree-balanced). Per tile: dma_gather of 512B rows for the tile's
    edges' sources, host-shipped transposed one-hot (STT, [dst-slot, edge])
    for the per-edge er matmul, device-built edge-major one-hot (ST3) for
    the aggregation + softmax-denominator matmul. Softmax runs without
    max-subtraction (logits are small); 1/denom applied per dst after
    aggregation, then bias + tanh + head-mean.
"""

import sys

import numpy as np

sys.path.insert(0, "/opt/trn_rl_repo")

import ml_dtypes

import concourse.bacc as bacc
import concourse.bass as bass
import concourse.mybir as mybir
from concourse.tile import TileContext

BF16 = mybir.dt.bfloat16
F8 = mybir.dt.float8e4
F32 = mybir.dt.float32
U8 = mybir.dt.uint8
I16 = mybir.dt.int16

P = 128
H = 4
D = 64
HD = H * D  # 256
ROWB = 512  # row: el 8B | er 8B | feat240 bf16 480B | feat16 fp8 16B
DSPLIT = 60  # feat dims [0, DSPLIT) bf16, [DSPLIT, 64) fp8
IN_DIM = 256
NEG_SLOPE = 0.2

N = 50000
E = 800000
NC = 8
N_PAD = 50176  # 8 * 49 * 128
NR = N_PAD // NC  # 6272 rows per core
TILES = NR // P  # 49 dst tiles per core
HALF = N_PAD // 2  # 25088 rows per table half (int16-indexable)
PAD_DSTLOC = 200.0  # any value >= 128: matches no dst slot
MAXC = 8  # dma_gather ucode caps at 1024 indices per instruction
SIM_INIT = False

NP_BF16 = ml_dtypes.bfloat16
NP_F8 = ml_dtypes.float8_e4m3


# --------------------------------------------------------------------------
# host-side preprocessing (index structures only; no float math on h/W)
# --------------------------------------------------------------------------

def _prep_core(dst_c, base):
    """Bin a core's dst nodes into TILES bins of P nodes balanced by
    in-degree. Returns member_ids [P, TILES] (global node ids), and per-edge
    (tile, pos)."""
    dst_local = dst_c - base
    indeg = np.bincount(dst_local, minlength=NR)
    order = np.argsort(-indeg, kind="stable")
    rounds = order.reshape(P, TILES).copy()  # snake-fill P rounds x TILES bins
    rounds[1::2] = rounds[1::2, ::-1]
    members = rounds
    tile_of = np.empty(NR, dtype=np.int64)
    pos_of = np.empty(NR, dtype=np.int64)
    tile_of[members.ravel()] = np.tile(np.arange(TILES), P)
    pos_of[members.ravel()] = np.repeat(np.arange(P), TILES)

    counts = indeg[members].sum(axis=0)
    tile_order = np.argsort(-counts, kind="stable")
    rank_of_tile = np.empty(TILES, dtype=np.int64)
    rank_of_tile[tile_order] = np.arange(TILES)

    member_ids = members[:, tile_order] + base  # [P, TILES] global ids
    t_e = rank_of_tile[tile_of[dst_local]]
    p_e = pos_of[dst_local]
    return member_ids, t_e, p_e


def preprocess(src, dst):
    src = np.asarray(src).astype(np.int64)
    dst = np.asarray(dst).astype(np.int64)
    core_of = dst // NR
    per_core = []
    lo_counts = np.zeros((NC, TILES), dtype=np.int64)
    hi_counts = np.zeros((NC, TILES), dtype=np.int64)
    for c in range(NC):
        m = core_of == c
        member_ids, t_e, p_e = _prep_core(dst[m], c * NR)
        # per-core permutation: local row r for node n
        #   rows [0, NR): my members, row t*P+p = member_ids[p, t]
        #   rows [NR, N_PAD): all other nodes in increasing id order
        perm = np.empty(N_PAD, dtype=np.int64)
        perm[:NR] = member_ids.T.reshape(-1)  # row t*P+p
        others = np.setdiff1d(np.arange(N_PAD), perm[:NR], assume_unique=False)
        perm[NR:] = others
        rowof = np.empty(N_PAD, dtype=np.int64)
        rowof[perm] = np.arange(N_PAD)
        r_e = rowof[src[m]]  # local table row of each edge's src
        is_lo = r_e < HALF
        per_core.append((r_e, member_ids, t_e, p_e, is_lo, perm))
        np.add.at(lo_counts[c], t_e[is_lo], 1)
        np.add.at(hi_counts[c], t_e[~is_lo], 1)
    clo = np.maximum(np.ceil(lo_counts.max(axis=0) / P).astype(np.int64), 1)
    chi = np.maximum(np.ceil(hi_counts.max(axis=0) / P).astype(np.int64), 1)
    c_tot = clo + chi
    # per-tile aux width (int16 cols): idx C*8 | dstloc C | STT C*128
    widths = c_tot * (8 + 1 + P)
    aux_offs = np.concatenate([[0], np.cumsum(widths)[:-1]])
    sum_w = int(widths.sum())

    one_bf = np.float32(1.0).astype(NP_BF16).view(np.int16)
    aux = []
    for c in range(NC):
        r_e, member_ids, t_e, p_e, is_lo, perm = per_core[c]
        auxw = np.zeros((P, sum_w), dtype=np.int16)
        # dstloc defaults to PAD (no one-hot match); STT defaults to 0
        for t in range(TILES):
            base = aux_offs[t] + c_tot[t] * 8
            auxw[:, base : base + c_tot[t]] = (
                np.full((P, c_tot[t]), PAD_DSTLOC, dtype=NP_BF16).view(np.int16)
            )
        for half in (True, False):
            sel = is_lo == half
            t_h = t_e[sel]
            s_h = r_e[sel] - (0 if half else HALF)
            p_h = p_e[sel]
            order = np.argsort(t_h, kind="stable")
            t_s, s_s, p_s = t_h[order], s_h[order], p_h[order]
            tile_starts = np.searchsorted(t_s, np.arange(TILES))
            q = np.arange(len(order)) - tile_starts[t_s]
            local_chunk = (0 if half else clo[t_s]) + q // P
            slot = q % P
            # gather idx: wrapped in 16 partitions
            icol = aux_offs[t_s] + local_chunk * 8 + slot // 16
            irow = slot % 16
            auxw[irow, icol] = s_s.astype(np.int16)
            # dstloc (bf16 bits) at [slot, C*8 + chunk]
            dcol = aux_offs[t_s] + c_tot[t_s] * 8 + local_chunk
            auxw[slot, dcol] = p_s.astype(NP_BF16).view(np.int16)
            # STT: partition = dst slot p_s, col = C*9 + chunk*128 + slot
            scol = aux_offs[t_s] + c_tot[t_s] * 9 + local_chunk * P + slot
            auxw[p_s, scol] = one_bf
        # replicate idx regions (16-wrapped) to all 8 groups of partitions
        for t in range(TILES):
            sl = slice(int(aux_offs[t]), int(aux_offs[t] + c_tot[t] * 8))
            auxw[:, sl] = np.tile(auxw[0:16, sl], (8, 1))
        aux.append(
            dict(
                auxw=auxw,
                member_ids=np.ascontiguousarray(member_ids.astype(np.int32)),
                perm=perm,
            )
        )
    return aux, [int(x) for x in clo], [int(x) for x in chi]


# --------------------------------------------------------------------------
# device kernel builder
# --------------------------------------------------------------------------

def build_kernel(n_pad, tiles, clo, chi):
    c_tot = [a + b for a, b in zip(clo, chi)]
    widths = [ct * (8 + 1 + P) for ct in c_tot]
    sum_w = int(sum(widths))
    half = n_pad // 2
    cmax = max(c_tot)
    nc = bacc.Bacc()

    hT = nc.declare_dram_parameter("hT", [IN_DIM, n_pad], BF16, isOutput=False)
    Wb = nc.declare_dram_parameter("Wb", [IN_DIM, HD], BF16, isOutput=False)
    WTb = nc.declare_dram_parameter("WTb", [IN_DIM, HD], BF16, isOutput=False)
    ALR = nc.declare_dram_parameter("ALR", [IN_DIM, 2 * H], BF16, isOutput=False)
    bias_dh = nc.declare_dram_parameter("bias_dh", [P, HD], F32, isOutput=False)
    iota_big = nc.declare_dram_parameter(
        "iota_big", [P, P * cmax], BF16, isOutput=False
    )
    auxw = nc.declare_dram_parameter("auxw", [P, sum_w], I16, isOutput=False)
    out = nc.declare_dram_parameter("out", [tiles * P, D], F32, isOutput=True)

    AL = mybir.AluOpType
    ACT = mybir.ActivationFunctionType
    KCH = IN_DIM // P  # 2 contraction chunks

    with TileContext(nc) as tc:
        with (
            tc.tile_pool(name="const", bufs=1) as constp,
            tc.tile_pool(name="dram", bufs=1, space="DRAM") as dramp,
        ):
            t_lo = dramp.tile([half, ROWB], U8)
            t_hi = dramp.tile([half, ROWB], U8)

            W_sb = constp.tile([P, KCH * HD], BF16)
            WT_sb = constp.tile([P, KCH * HD], BF16)
            ALR_sb = constp.tile([P, KCH * 2 * H], BF16)
            WALR_sb = constp.tile([P, KCH * 2 * H], BF16)
            bias_sb = constp.tile([P, HD], F32)
            iota_sb = constp.tile([P, P * cmax], BF16)
            for kk in range(KCH):
                nc.sync.dma_start(
                    out=W_sb[:, kk * HD : (kk + 1) * HD],
                    in_=Wb[kk * P : (kk + 1) * P, :],
                )
                nc.sync.dma_start(
                    out=WT_sb[:, kk * HD : (kk + 1) * HD],
                    in_=WTb[kk * P : (kk + 1) * P, :],
                )
                nc.sync.dma_start(
                    out=ALR_sb[:, kk * 2 * H : (kk + 1) * 2 * H],
                    in_=ALR[kk * P : (kk + 1) * P, :],
                )
            nc.sync.dma_start(out=bias_sb[:], in_=bias_dh[:, :])
            nc.sync.dma_start(out=iota_sb[:], in_=iota_big[:, :])

            # WALR = W @ ALR
            with tc.tile_pool(name="setup_ps", bufs=1, space="PSUM") as setupps:
                for ic in range(KCH):
                    walr_ps = setupps.tile([P, 2 * H], F32)
                    for kk in range(KCH):
                        nc.tensor.matmul(
                            walr_ps[:],
                            lhsT=WT_sb[:, kk * HD + ic * P : kk * HD + (ic + 1) * P],
                            rhs=ALR_sb[:, kk * 2 * H : (kk + 1) * 2 * H],
                            start=(kk == 0),
                            stop=(kk == KCH - 1),
                        )
                    nc.vector.tensor_copy(
                        out=WALR_sb[:, ic * 2 * H : (ic + 1) * 2 * H], in_=walr_ps[:]
                    )

            # ------------------- phase 1: projection table -------------------
            OB = 1024  # rows per outer block
            SUBS = OB // P
            n_ob = n_pad // OB
            with (
                tc.tile_pool(name="p1", bufs=3) as p1,
                tc.tile_pool(name="p1ps", bufs=3, space="PSUM") as p1ps,
            ):
                for ob in range(n_ob):
                    start = ob * OB
                    hT_t = p1.tile([P, KCH, OB], BF16, name="hT_t", tag="hT_t")
                    for kk in range(KCH):
                        nc.sync.dma_start(
                            out=hT_t[:, kk, :],
                            in_=hT[kk * P : (kk + 1) * P, start : start + OB],
                        )
                    stage = p1.tile([P, SUBS, ROWB], U8, name="stage", tag="stage")
                    for sub in range(SUBS):
                        feat_ps = p1ps.tile([P, HD], F32, name="feat_ps", tag="feat_ps")
                        elr_ps = p1ps.tile([P, 2 * H], F32, name="elr_ps", tag="elr_ps")
                        for kk in range(KCH):
                            lh = hT_t[:, kk, sub * P : (sub + 1) * P]
                            nc.tensor.matmul(
                                feat_ps[:],
                                lhsT=lh,
                                rhs=W_sb[:, kk * HD : (kk + 1) * HD],
                                start=(kk == 0),
                                stop=(kk == KCH - 1),
                            )
                            nc.tensor.matmul(
                                elr_ps[:],
                                lhsT=lh,
                                rhs=WALR_sb[:, kk * 2 * H : (kk + 1) * 2 * H],
                                start=(kk == 0),
                                stop=(kk == KCH - 1),
                            )
                        # elr -> row bytes [0,16) as bf16 (el 0:4, er 4:8)
                        nc.vector.tensor_copy(
                            out=stage[:, sub, 0:16].bitcast(BF16), in_=elr_ps[:]
                        )
                        # feat dims d<60 -> bytes [16,496) bf16, (d,h)-major
                        fview = feat_ps[:].rearrange("p (h d) -> p d h", h=H)
                        big_out = (
                            stage[:, sub, 16 : 16 + 2 * DSPLIT * H]
                            .bitcast(BF16)
                            .rearrange("p (d h) -> p d h", h=H)
                        )
                        if sub % 2 == 0:
                            nc.vector.tensor_copy(out=big_out, in_=fview[:, 0:DSPLIT, :])
                        else:
                            nc.scalar.copy(out=big_out, in_=fview[:, 0:DSPLIT, :])
                        # feat dims d>=60 -> bytes [496,512) fp8, (d,h)-major
                        nc.scalar.copy(
                            out=stage[:, sub, 16 + 2 * DSPLIT * H : ROWB]
                            .bitcast(F8)
                            .rearrange("p (d h) -> p d h", h=H),
                            in_=fview[:, DSPLIT:D, :],
                        )
                    ranges = []
                    if start + OB <= half:
                        ranges.append((0, SUBS, t_lo, start))
                    elif start >= half:
                        ranges.append((0, SUBS, t_hi, start - half))
                    else:
                        sub_split = (half - start) // P
                        ranges.append((0, sub_split, t_lo, start))
                        ranges.append((sub_split, SUBS, t_hi, 0))
                    for s0, s1, tgt, r0 in ranges:
                        nsub = s1 - s0
                        dst_ap = tgt[r0 : r0 + nsub * P, :].rearrange(
                            "(s p) c -> p s c", p=P
                        )
                        nc.sync.dma_start(out=dst_ap, in_=stage[:, s0:s1, :])

            # ------------------- phase 2: edge aggregation -------------------
            with (
                tc.tile_pool(name="p2", bufs=3) as p2,
                tc.tile_pool(name="p2s", bufs=4) as p2s,
                tc.tile_pool(name="outps", bufs=2, space="PSUM") as outps_pool,
                tc.tile_pool(name="ergps", bufs=2, space="PSUM") as ergps_pool,
            ):
                aux_off = 0
                for t in range(tiles):
                    C = int(c_tot[t])
                    W_t = C * (8 + 1 + P)
                    aux_t = p2.tile([P, W_t], I16, name="aux_t", tag="aux")
                    nc.sync.dma_start(
                        out=aux_t[:], in_=auxw[:, aux_off : aux_off + W_t]
                    )
                    idx_v = aux_t[:, 0 : C * 8]
                    dl_v = aux_t[:, C * 8 : C * 9].bitcast(BF16)
                    stt_v = (
                        aux_t[:, C * 9 : C * 9 + C * P]
                        .bitcast(BF16)
                        .rearrange("p (c e) -> p c e", c=C)
                    )

                    # member el/er (rows t*P..t*P+P of MY table = t_lo region)
                    ert = p2s.tile([P, 16], U8, name="ert", tag="ert")
                    nc.sync.dma_start(
                        out=ert[:], in_=t_lo[t * P : (t + 1) * P, 0:16]
                    )
                    er_t = ert[:, 8:16].bitcast(BF16)  # [P(d), H]

                    G = p2.tile([P, C, ROWB], U8, name="G", tag="G")
                    for base, width, tb in ((0, int(clo[t]), t_lo), (int(clo[t]), int(chi[t]), t_hi)):
                        done = 0
                        while done < width:
                            w = min(MAXC, width - done)
                            b = base + done
                            nc.gpsimd.dma_gather(
                                out_ap=G[:, b : b + w, :],
                                in_ap=tb[:, :],
                                idxs_ap=idx_v[:, b * 8 : (b + w) * 8],
                                num_idxs=w * P,
                                num_idxs_reg=w * P,
                                elem_size=ROWB,
                            )
                            done += w

                    # edge-major one-hot ST3[e, d, j] = (dstloc[e, j] == d)
                    ST3 = p2.tile([P, P, C], BF16, name="ST3", tag="ST")
                    nc.vector.tensor_tensor(
                        out=ST3[:],
                        in0=dl_v.rearrange("p (one c) -> p one c", one=1).to_broadcast(
                            [P, P, C]
                        ),
                        in1=iota_sb[:].rearrange("p (d c) -> p d c", d=P)[:, :, 0:C],
                        op=AL.is_equal,
                    )

                    # er per edge: erg[e, j, h] = sum_d STT[d, j, e] er_t[d, h]
                    erg_ps = ergps_pool.tile([P, C, H], F32, name="erg_ps")
                    for j in range(C):
                        nc.tensor.matmul(
                            erg_ps[:, j, :],
                            lhsT=stt_v[:, j, :],
                            rhs=er_t,
                            start=True,
                            stop=True,
                        )

                    # ev = el[src] + er[dst]; lrel = leaky_relu(ev)
                    ev = p2s.tile([P, C, H], F32, name="ev", tag="ev")
                    nc.vector.tensor_tensor(
                        out=ev[:],
                        in0=G[:, :, 0:8].bitcast(BF16)[:, :, 0:H],
                        in1=erg_ps[:],
                        op=AL.add,
                    )
                    lrel = p2s.tile([P, C, H], F32, name="lrel", tag="lrel")
                    nc.vector.scalar_tensor_tensor(
                        out=lrel[:],
                        in0=ev[:],
                        scalar=NEG_SLOPE,
                        in1=ev[:],
                        op0=AL.mult,
                        op1=AL.max,
                    )
                    exb = p2s.tile([P, C, H], BF16, name="exb", tag="exb")
                    nc.scalar.activation(out=exb[:], in_=lrel[:], func=ACT.Exp)

                    # gx chunk layout: [ ex*feat240 | ex*feat16 | ex (H) ]
                    GXW = HD + H  # 260
                    gx = p2.tile([P, C, GXW], BF16, name="gx", tag="gx")
                    nc.vector.tensor_copy(out=gx[:, :, HD : HD + H], in_=exb[:])
                    exbc = exb[:].rearrange("p c (one h) -> p c one h", one=1)
                    nc.vector.tensor_tensor(
                        out=gx[:, :, 0 : DSPLIT * H].rearrange(
                            "p c (d h) -> p c d h", h=H
                        ),
                        in0=G[:, :, 16 : 16 + 2 * DSPLIT * H]
                        .bitcast(BF16)
                        .rearrange("p c (d h) -> p c d h", h=H),
                        in1=exbc.to_broadcast([P, C, DSPLIT, H]),
                        op=AL.mult,
                    )
                    ftail = p2s.tile([P, C, (D - DSPLIT) * H], BF16, name="ft", tag="ft")
                    nc.scalar.activation(
                        out=ftail[:],
                        in_=G[:, :, 16 + 2 * DSPLIT * H : ROWB].bitcast(F8),
                        func=ACT.Copy,
                    )
                    nc.vector.tensor_tensor(
                        out=gx[:, :, DSPLIT * H : HD].rearrange(
                            "p c (d h) -> p c d h", h=H
                        ),
                        in0=ftail[:].rearrange("p c (d h) -> p c d h", h=H),
                        in1=exbc.to_broadcast([P, C, D - DSPLIT, H]),
                        op=AL.mult,
                    )

                    out_ps = outps_pool.tile([P, GXW], F32, name="out_ps")
                    for j in range(C):
                        nc.tensor.matmul(
                            out_ps[:],
                            lhsT=ST3[:, :, j],
                            rhs=gx[:, j, :],
                            start=(j == 0),
                            stop=(j == C - 1),
                        )

                    # epilogue: normalize, bias, tanh, mean over heads
                    rd0 = p2s.tile([P, H], F32, name="rd0", tag="rd0")
                    nc.vector.tensor_scalar(
                        out=rd0[:],
                        in0=out_ps[:, HD : HD + H],
                        scalar1=1e-9,
                        scalar2=None,
                        op0=AL.max,
                    )
                    rd = p2s.tile([P, H], F32, name="rd", tag="rd")
                    nc.vector.reciprocal(out=rd[:], in_=rd0[:])
                    nrm = p2.tile([P, HD], F32, name="nrm", tag="nrm")
                    nc.vector.tensor_tensor(
                        out=nrm[:].rearrange("p (d h) -> p d h", h=H),
                        in0=out_ps[:, 0:HD].rearrange("p (d h) -> p d h", h=H),
                        in1=rd[:]
                        .rearrange("p (one h) -> p one h", one=1)
                        .to_broadcast([P, D, H]),
                        op=AL.mult,
                    )
                    nb = p2.tile([P, HD], F32, name="nb", tag="nb")
                    nc.gpsimd.tensor_tensor(
                        out=nb[:], in0=nrm[:], in1=bias_sb[:], op=AL.add
                    )
                    th = p2.tile([P, HD], F32, name="th", tag="th")
                    nc.scalar.activation(out=th[:], in_=nb[:], func=ACT.Tanh)
                    thv = th[:].rearrange("p (d h) -> p d h", h=H)
                    m1 = p2s.tile([P, D], F32, name="m1", tag="m1")
                    nc.vector.tensor_tensor(
                        out=m1[:], in0=thv[:, :, 0], in1=thv[:, :, 1], op=AL.add
                    )
                    m2 = p2s.tile([P, D], F32, name="m2", tag="m2")
                    nc.gpsimd.tensor_tensor(
                        out=m2[:], in0=thv[:, :, 2], in1=thv[:, :, 3], op=AL.add
                    )
                    m3 = p2s.tile([P, D], F32, name="m3", tag="m3")
                    nc.vector.tensor_tensor(out=m3[:], in0=m1[:], in1=m2[:], op=AL.add)
                    of = p2.tile([P, D], F32, name="of", tag="of")
                    nc.vector.tensor_scalar(
                        out=of[:], in0=m3[:], scalar1=0.25, scalar2=None, op0=AL.mult
                    )
                    nc.sync.dma_start(out=out[t * P : (t + 1) * P, :], in_=of[:])
                    aux_off += W_t
    return nc


# --------------------------------------------------------------------------
# host entry
# --------------------------------------------------------------------------

def _make_static_inputs(W, attn_l, attn_r, bias):
    bf = NP_BF16
    Wb = np.asarray(W, dtype=np.float32).astype(bf)
    WTb = np.ascontiguousarray(np.asarray(W, dtype=np.float32).T).astype(bf)
    ALRm = np.zeros((IN_DIM, 2 * H), dtype=np.float32)
    al = np.asarray(attn_l, dtype=np.float32)
    ar = np.asarray(attn_r, dtype=np.float32)
    for hh in range(H):
        ALRm[hh * D : (hh + 1) * D, hh] = al[hh]
        ALRm[hh * D : (hh + 1) * D, H + hh] = ar[hh]
    ALRm = ALRm.astype(bf)
    # bias in (d,h)-major layout
    b = np.asarray(bias, dtype=np.float32).reshape(H, D)
    bias_dh = np.ascontiguousarray(b.T.reshape(1, HD))
    bias_rep = np.tile(bias_dh, (P, 1))
    return dict(Wb=Wb, WTb=WTb, ALR=ALRm, bias_dh=np.ascontiguousarray(bias_rep))


def make_iota_big(cmax):
    row = np.repeat(np.arange(P, dtype=np.float32), cmax).reshape(1, P * cmax)
    return np.ascontiguousarray(np.tile(row, (P, 1)).astype(NP_BF16))


def bench(nc, in_maps, n_iters=10):
    """Repeated-execution wall timing of the compiled SPMD kernel via PJRT."""
    import time

    import jax
    from jax.sharding import Mesh, NamedSharding, PartitionSpec
    from jax.experimental.shard_map import shard_map

    from concourse import bass2jax, mybir as _mb

    bass2jax.install_neuronx_cc_hook()
    n_cores = len(in_maps)
    in_names, out_names, out_avals, zero_outs = [], [], [], []
    partition_name = nc.partition_id_tensor.name if nc.partition_id_tensor else None
    for alloc in nc.m.functions[0].allocations:
        if not isinstance(alloc, _mb.MemoryLocationSet):
            continue
        name = alloc.memorylocations[0].name
        if alloc.kind == "ExternalInput":
            if name != partition_name:
                in_names.append(name)
        elif alloc.kind == "ExternalOutput":
            out_names.append(name)
            shape = tuple(alloc.tensor_shape)
            dtype = _mb.dt.np(alloc.dtype)
            out_avals.append(jax.core.ShapedArray(shape, dtype))
            zero_outs.append(np.zeros(shape, dtype))
    n_params = len(in_names)
    all_in_names = in_names + out_names
    if partition_name is not None:
        all_in_names.append(partition_name)

    def _body(*args):
        operands = list(args)
        if partition_name is not None:
            operands.append(bass2jax.partition_id_tensor())
        outs = bass2jax._bass_exec_p.bind(
            *operands,
            out_avals=tuple(out_avals),
            in_names=tuple(all_in_names),
            out_names=tuple(out_names),
            lowering_input_output_aliases=(),
            sim_require_finite=True,
            sim_require_nnan=True,
            nc=nc,
        )
        return tuple(outs)

    devices = jax.devices()[:n_cores]
    mesh = Mesh(np.asarray(devices), ("core",))
    n_outs = len(out_names)
    sharded = jax.jit(
        shard_map(
            _body,
            mesh=mesh,
            in_specs=(PartitionSpec("core"),) * (n_params + n_outs),
            out_specs=(PartitionSpec("core"),) * n_outs,
            check_rep=False,
        ),
        keep_unused=True,
    )
    sh = NamedSharding(mesh, PartitionSpec("core"))
    concat_in = [
        jax.device_put(
            np.concatenate([np.asarray(in_maps[c][nm]) for c in range(n_cores)], 0), sh
        )
        for nm in in_names
    ]
    concat_zeros = [
        jax.device_put(np.zeros((n_cores * z.shape[0], *z.shape[1:]), z.dtype), sh)
        for z in zero_outs
    ]
    outs = sharded(*concat_in, *concat_zeros)  # warmup/compile
    jax.block_until_ready(outs)
    times = []
    for _ in range(n_iters):
        t0 = time.perf_counter()
        outs = sharded(*concat_in, *concat_zeros)
        jax.block_until_ready(outs)
        times.append(time.perf_counter() - t0)
    results = [
        {
            nm: np.asarray(outs[i]).reshape(n_cores, *out_avals[i].shape)[c]
            for i, nm in enumerate(out_names)
        }
        for c in range(n_cores)
    ]
    return times, results


def kernel(h, W, attn_l, attn_r, bias, src, dst):
    from concourse.bass_utils import run_bass_kernel_spmd

    aux, clo, chi = preprocess(src, dst)
    static = _make_static_inputs(W, attn_l, attn_r, bias)
    nc = build_kernel(N_PAD, TILES, clo, chi)
    nc.compile()
    iota_big = make_iota_big(max(a + b for a, b in zip(clo, chi)))
    h_pad = np.zeros((N_PAD, IN_DIM), dtype=np.float32)
    h_pad[:N] = np.asarray(h, dtype=np.float32)
    in_maps = []
    for c in range(NC):
        m = dict(static)
        m["hT"] = np.ascontiguousarray(h_pad[aux[c]["perm"]].T).astype(NP_BF16)
        m["auxw"] = aux[c]["auxw"]
        m["iota_big"] = iota_big
        in_maps.append(m)
    res = run_bass_kernel_spmd(nc, in_maps, core_ids=list(range(NC)), trace=False)
    out_full = np.zeros((N, D), dtype=np.float32)
    for c in range(NC):
        dev = res.results[c]["out"]  # [TILES*P, D]
        ids = aux[c]["member_ids"]  # [P, TILES]
        rows = ids.T.reshape(-1)  # row t*P+p  <->  ids[p, t]
        valid = rows < N
        out_full[rows[valid]] = dev[valid]
    kernel.last_nc = nc
    kernel.last_in_maps = in_maps
    kernel.last_aux = aux
    return out_full


# revision 7
# speedup vs baseline: 2.2184x; 1.9962x over previous
"""GAT layer (AdaptiveBreadthLayer) on 8 TRN2 NeuronCores.

Strategy (v3):
  - dst-shard: core c owns destination nodes [c*6272, (c+1)*6272) (N padded
    50000 -> 50176). Every edge lives on exactly one core (by dst): no
    collectives.
  - Each core redundantly computes a full projection table with PER-CORE
    PERMUTED row order (its own member nodes first, in (tile,pos) order) so
    member er values come from one tiny contiguous DMA at SPMD-uniform
    addresses. Rows are 512B (the DMA-gather sweet spot):
      {el 4xbf16 | er 4xbf16 | feat dims d<60 (d,h)-major bf16 (480B)
       | feat dims d>=60 (d,h)-major fp8e4m3 (16B)}
    (mixed precision keeps rel err ~7e-3, well under the 2e-2 gate).
    The (d,h)-major layout makes the per-edge exp-weighting multiply a
    packed-bf16 DVE op (2x mode) with the broadcast on a middle dim.
  - Phase 2 walks the core's 49 destination tiles (128 dst nodes each,
    degree-balanced). Per tile: dma_gather of 512B rows for the tile's
    edges' sources; host-shipped fp8 one-hot matrices in BOTH orientations
    (edge-major ST3 for the aggregation + softmax-denominator matmul,
    dst-major STT for the per-edge er matmul) feed mixed-dtype matmuls
    directly - nothing is built on the vector engine. Softmax runs without
    max-subtraction (logits are small); 1/denom applied per dst after
    aggregation, then bias + tanh + head-mean.
"""

import sys

import numpy as np

sys.path.insert(0, "/opt/trn_rl_repo")

import ml_dtypes

import concourse.bacc as bacc
import concourse.bass as bass
import concourse.mybir as mybir
from concourse.tile import TileContext

BF16 = mybir.dt.bfloat16
F8 = mybir.dt.float8e4
F32 = mybir.dt.float32
U8 = mybir.dt.uint8
I16 = mybir.dt.int16

P = 128
H = 4
D = 64
HD = H * D  # 256
ROWB = 512  # row: el 8B | er 8B | feat240 bf16 480B | feat16 fp8 16B
DSPLIT = 60  # feat dims [0, DSPLIT) bf16, [DSPLIT, 64) fp8
IN_DIM = 256
NEG_SLOPE = 0.2

N = 50000
E = 800000
NC = 8
N_PAD = 50176  # 8 * 49 * 128
NR = N_PAD // NC  # 6272 rows per core
TILES = NR // P  # 49 dst tiles per core
HALF = N_PAD // 2  # 25088 rows per table half (int16-indexable)
MAXC = 8  # dma_gather ucode caps at 1024 indices per instruction
SIM_INIT = False

NP_BF16 = ml_dtypes.bfloat16
NP_F8 = ml_dtypes.float8_e4m3
F8_ONE = np.float32(1.0).astype(NP_F8).tobytes()[0]  # fp8e4m3 bits of 1.0


# --------------------------------------------------------------------------
# host-side preprocessing (index structures only; no float math on h/W)
# --------------------------------------------------------------------------

def _prep_core(dst_c, base):
    """Bin a core's dst nodes into TILES bins of P nodes balanced by
    in-degree."""
    dst_local = dst_c - base
    indeg = np.bincount(dst_local, minlength=NR)
    order = np.argsort(-indeg, kind="stable")
    rounds = order.reshape(P, TILES).copy()  # snake-fill P rounds x TILES bins
    rounds[1::2] = rounds[1::2, ::-1]
    members = rounds
    tile_of = np.empty(NR, dtype=np.int64)
    pos_of = np.empty(NR, dtype=np.int64)
    tile_of[members.ravel()] = np.tile(np.arange(TILES), P)
    pos_of[members.ravel()] = np.repeat(np.arange(P), TILES)

    counts = indeg[members].sum(axis=0)
    tile_order = np.argsort(-counts, kind="stable")
    rank_of_tile = np.empty(TILES, dtype=np.int64)
    rank_of_tile[tile_order] = np.arange(TILES)

    member_ids = members[:, tile_order] + base  # [P, TILES] global ids
    t_e = rank_of_tile[tile_of[dst_local]]
    p_e = pos_of[dst_local]
    return member_ids, t_e, p_e


def preprocess(src, dst):
    src = np.asarray(src).astype(np.int64)
    dst = np.asarray(dst).astype(np.int64)
    core_of = dst // NR
    per_core = []
    lo_counts = np.zeros((NC, TILES), dtype=np.int64)
    hi_counts = np.zeros((NC, TILES), dtype=np.int64)
    for c in range(NC):
        m = core_of == c
        member_ids, t_e, p_e = _prep_core(dst[m], c * NR)
        # per-core permutation: local row r holds node perm[r]
        #   rows [0, NR): my members, row t*P+p = member_ids[p, t]
        #   rows [NR, N_PAD): all other nodes in increasing id order
        perm = np.empty(N_PAD, dtype=np.int64)
        perm[:NR] = member_ids.T.reshape(-1)
        perm[NR:] = np.setdiff1d(np.arange(N_PAD), perm[:NR])
        rowof = np.empty(N_PAD, dtype=np.int64)
        rowof[perm] = np.arange(N_PAD)
        r_e = rowof[src[m]]  # local table row of each edge's src
        is_lo = r_e < HALF
        per_core.append((r_e, member_ids, t_e, p_e, is_lo, perm))
        np.add.at(lo_counts[c], t_e[is_lo], 1)
        np.add.at(hi_counts[c], t_e[~is_lo], 1)
    clo = np.maximum(np.ceil(lo_counts.max(axis=0) / P).astype(np.int64), 1)
    chi = np.maximum(np.ceil(hi_counts.max(axis=0) / P).astype(np.int64), 1)
    c_tot = clo + chi
    # per-tile aux bytes: idx C*16 | ST3 C*128 (fp8) | STT C*128 (fp8)
    widths_b = c_tot * (16 + P + P)
    aux_offs = np.concatenate([[0], np.cumsum(widths_b)[:-1]])
    sum_b = int(widths_b.sum())

    aux = []
    for c in range(NC):
        r_e, member_ids, t_e, p_e, is_lo, perm = per_core[c]
        auxb = np.zeros((P, sum_b), dtype=np.uint8)
        for half in (True, False):
            sel = is_lo == half
            t_h = t_e[sel]
            s_h = r_e[sel] - (0 if half else HALF)
            p_h = p_e[sel]
            order = np.argsort(t_h, kind="stable")
            t_s, s_s, p_s = t_h[order], s_h[order], p_h[order]
            tile_starts = np.searchsorted(t_s, np.arange(TILES))
            q = np.arange(len(order)) - tile_starts[t_s]
            local_chunk = (0 if half else clo[t_s]) + q // P
            slot = q % P
            # gather idx int16 at byte col aux_off + chunk*16 + (slot//16)*2,
            # partition slot%16 (16-wrapped), replicated to 8 groups below
            icol = aux_offs[t_s] + local_chunk * 16 + (slot // 16) * 2
            irow = slot % 16
            i16 = s_s.astype(np.int16)
            auxb[irow, icol] = (i16 & 0xFF).astype(np.uint8)
            auxb[irow, icol + 1] = ((i16 >> 8) & 0xFF).astype(np.uint8)
            # ST3 (edge-major): partition = slot, col = chunk*128 + dstslot
            s3col = aux_offs[t_s] + c_tot[t_s] * 16 + local_chunk * P + p_s
            auxb[slot, s3col] = F8_ONE
            # STT (dst-major): partition = dstslot, col = chunk*128 + slot
            stcol = aux_offs[t_s] + c_tot[t_s] * (16 + P) + local_chunk * P + slot
            auxb[p_s, stcol] = F8_ONE
        # replicate idx regions (16-wrapped) to all 8 partition groups
        for t in range(TILES):
            sl = slice(int(aux_offs[t]), int(aux_offs[t] + c_tot[t] * 16))
            auxb[:, sl] = np.tile(auxb[0:16, sl], (8, 1))
        aux.append(
            dict(
                auxw=auxb.view(np.int16),
                member_ids=np.ascontiguousarray(member_ids.astype(np.int32)),
                perm=perm,
            )
        )
    return aux, [int(x) for x in clo], [int(x) for x in chi]


# --------------------------------------------------------------------------
# device kernel builder
# --------------------------------------------------------------------------

def build_kernel(n_pad, tiles, clo, chi):
    c_tot = [a + b for a, b in zip(clo, chi)]
    widths = [ct * (8 + P) for ct in c_tot]  # int16 cols per tile
    sum_w = int(sum(widths))
    half = n_pad // 2
    nc = bacc.Bacc()

    hT = nc.declare_dram_parameter("hT", [IN_DIM, n_pad], BF16, isOutput=False)
    # WCAT: [W (256) | W^T (256) | ALR (8)] along columns
    WCAT = nc.declare_dram_parameter("WCAT", [IN_DIM, 2 * HD + 2 * H], BF16,
                                     isOutput=False)
    bias_dh = nc.declare_dram_parameter("bias_dh", [P, HD], F32, isOutput=False)
    auxw = nc.declare_dram_parameter("auxw", [P, sum_w], I16, isOutput=False)
    out = nc.declare_dram_parameter("out", [tiles * P, D], F32, isOutput=True)

    AL = mybir.AluOpType
    ACT = mybir.ActivationFunctionType
    KCH = IN_DIM // P  # 2 contraction chunks
    WW = 2 * HD + 2 * H  # 520

    with TileContext(nc) as tc:
        with (
            tc.tile_pool(name="const", bufs=1) as constp,
            tc.tile_pool(name="dram", bufs=1, space="DRAM") as dramp,
        ):
            t_lo = dramp.tile([half, ROWB], U8)
            t_hi = dramp.tile([half, ROWB], U8)

            wcat_sb = constp.tile([P, KCH, WW], BF16)
            bias_sb = constp.tile([P, HD], F32)
            WALR_sb = constp.tile([P, KCH * 2 * H], BF16)
            nc.sync.dma_start(
                out=wcat_sb[:],
                in_=WCAT[:, :].rearrange("(k p) c -> p k c", p=P),
            )
            nc.sync.dma_start(out=bias_sb[:], in_=bias_dh[:, :])
            W_s = lambda kk: wcat_sb[:, kk, 0:HD]
            WT_s = lambda kk: wcat_sb[:, kk, HD : 2 * HD]
            ALR_s = lambda kk: wcat_sb[:, kk, 2 * HD : WW]

            # WALR = W @ ALR
            with tc.tile_pool(name="setup_ps", bufs=1, space="PSUM") as setupps:
                for ic in range(KCH):
                    walr_ps = setupps.tile([P, 2 * H], F32)
                    for kk in range(KCH):
                        nc.tensor.matmul(
                            walr_ps[:],
                            lhsT=WT_s(kk)[:, ic * P : (ic + 1) * P],
                            rhs=ALR_s(kk),
                            start=(kk == 0),
                            stop=(kk == KCH - 1),
                        )
                    nc.vector.tensor_copy(
                        out=WALR_sb[:, ic * 2 * H : (ic + 1) * 2 * H], in_=walr_ps[:]
                    )

            # ------------------- phase 1: projection table -------------------
            OB = 1024  # rows per outer block
            SUBS = OB // P
            n_ob = n_pad // OB
            with (
                tc.tile_pool(name="p1", bufs=3) as p1,
                tc.tile_pool(name="p1ps", bufs=3, space="PSUM") as p1ps,
            ):
                for ob in range(n_ob):
                    start = ob * OB
                    hT_t = p1.tile([P, KCH, OB], BF16, name="hT_t", tag="hT_t")
                    nc.sync.dma_start(
                        out=hT_t[:],
                        in_=hT[:, start : start + OB].rearrange(
                            "(k p) n -> p k n", p=P
                        ),
                    )
                    stage = p1.tile([P, SUBS, ROWB], U8, name="stage", tag="stage")
                    for sub in range(SUBS):
                        feat_ps = p1ps.tile([P, HD], F32, name="feat_ps", tag="feat_ps")
                        elr_ps = p1ps.tile([P, 2 * H], F32, name="elr_ps", tag="elr_ps")
                        for kk in range(KCH):
                            lh = hT_t[:, kk, sub * P : (sub + 1) * P]
                            nc.tensor.matmul(
                                feat_ps[:],
                                lhsT=lh,
                                rhs=W_s(kk),
                                start=(kk == 0),
                                stop=(kk == KCH - 1),
                            )
                            nc.tensor.matmul(
                                elr_ps[:],
                                lhsT=lh,
                                rhs=WALR_sb[:, kk * 2 * H : (kk + 1) * 2 * H],
                                start=(kk == 0),
                                stop=(kk == KCH - 1),
                            )
                        # elr -> row bytes [0,16) as bf16 (el 0:4, er 4:8)
                        nc.vector.tensor_copy(
                            out=stage[:, sub, 0:16].bitcast(BF16), in_=elr_ps[:]
                        )
                        # feat dims d<60 -> bytes [16,496) bf16, (d,h)-major
                        fview = feat_ps[:].rearrange("p (h d) -> p d h", h=H)
                        big_out = (
                            stage[:, sub, 16 : 16 + 2 * DSPLIT * H]
                            .bitcast(BF16)
                            .rearrange("p (d h) -> p d h", h=H)
                        )
                        if sub % 4 != 3:
                            nc.vector.tensor_copy(out=big_out, in_=fview[:, 0:DSPLIT, :])
                        else:
                            nc.scalar.copy(out=big_out, in_=fview[:, 0:DSPLIT, :])
                        # feat dims d>=60 -> bytes [496,512) fp8, (d,h)-major
                        nc.scalar.copy(
                            out=stage[:, sub, 16 + 2 * DSPLIT * H : ROWB]
                            .bitcast(F8)
                            .rearrange("p (d h) -> p d h", h=H),
                            in_=fview[:, DSPLIT:D, :],
                        )
                    ranges = []
                    if start + OB <= half:
                        ranges.append((0, SUBS, t_lo, start))
                    elif start >= half:
                        ranges.append((0, SUBS, t_hi, start - half))
                    else:
                        sub_split = (half - start) // P
                        ranges.append((0, sub_split, t_lo, start))
                        ranges.append((sub_split, SUBS, t_hi, 0))
                    for s0, s1, tgt, r0 in ranges:
                        nsub = s1 - s0
                        dst_ap = tgt[r0 : r0 + nsub * P, :].rearrange(
                            "(s p) c -> p s c", p=P
                        )
                        nc.sync.dma_start(out=dst_ap, in_=stage[:, s0:s1, :])

            # ------------------- phase 2: edge aggregation -------------------
            with (
                tc.tile_pool(name="p2", bufs=4) as p2,
                tc.tile_pool(name="p2g", bufs=3) as p2g,
                tc.tile_pool(name="p2s", bufs=6) as p2s,
                tc.tile_pool(name="outps", bufs=2, space="PSUM") as outps_pool,
                tc.tile_pool(name="ergps", bufs=2, space="PSUM") as ergps_pool,
            ):
                # member el/er for all tiles in one strided DMA
                ert_all = constp.tile([P, tiles, 16], U8)
                nc.sync.dma_start(
                    out=ert_all[:],
                    in_=t_lo[0 : tiles * P, 0:16].rearrange("(t p) c -> p t c", p=P),
                )
                of_all = constp.tile([P, tiles, D], F32)

                aux_off = 0
                for t in range(tiles):
                    C = int(c_tot[t])
                    W_t = C * (8 + P)
                    aux_t = p2.tile([P, W_t], I16, name="aux_t", tag="aux")
                    nc.sync.dma_start(
                        out=aux_t[:], in_=auxw[:, aux_off : aux_off + W_t]
                    )
                    idx_v = aux_t[:, 0 : C * 8]
                    st3_v = (
                        aux_t[:, C * 8 : C * 72]
                        .bitcast(F8)
                        .rearrange("p (c e) -> p c e", c=C)
                    )
                    stt_v = (
                        aux_t[:, C * 72 : C * 136]
                        .bitcast(F8)
                        .rearrange("p (c e) -> p c e", c=C)
                    )
                    er_t = ert_all[:, t, 8:16].bitcast(BF16)  # [P(d), H]

                    G = p2g.tile([P, C, ROWB], U8, name="G", tag="G")
                    for base, width, tb in (
                        (0, int(clo[t]), t_lo),
                        (int(clo[t]), int(chi[t]), t_hi),
                    ):
                        done = 0
                        while done < width:
                            w = min(MAXC, width - done)
                            b = base + done
                            nc.gpsimd.dma_gather(
                                out_ap=G[:, b : b + w, :],
                                in_ap=tb[:, :],
                                idxs_ap=idx_v[:, b * 8 : (b + w) * 8],
                                num_idxs=w * P,
                                num_idxs_reg=w * P,
                                elem_size=ROWB,
                            )
                            done += w

                    # er per edge: erg[e, j, h] = sum_d STT[d, j, e] er_t[d, h]
                    erg_ps = ergps_pool.tile([P, C, H], F32, name="erg_ps")
                    for j in range(C):
                        nc.tensor.matmul(
                            erg_ps[:, j, :],
                            lhsT=stt_v[:, j, :],
                            rhs=er_t,
                            start=True,
                            stop=True,
                        )

                    # ev = el[src] + er[dst]; lrel = leaky_relu(ev); ex = exp
                    ev = p2s.tile([P, C, H], F32, name="ev", tag="ev")
                    nc.vector.tensor_tensor(
                        out=ev[:],
                        in0=G[:, :, 0:8].bitcast(BF16)[:, :, 0:H],
                        in1=erg_ps[:],
                        op=AL.add,
                    )
                    lrel = p2s.tile([P, C, H], F32, name="lrel", tag="lrel")
                    nc.vector.scalar_tensor_tensor(
                        out=lrel[:],
                        in0=ev[:],
                        scalar=NEG_SLOPE,
                        in1=ev[:],
                        op0=AL.mult,
                        op1=AL.max,
                    )
                    exb = p2s.tile([P, C, H], BF16, name="exb", tag="exb")
                    nc.scalar.activation(out=exb[:], in_=lrel[:], func=ACT.Exp)

                    # gx chunk layout: [ ex*feat240 | ex*feat16 | ex (H) ]
                    GXW = HD + H  # 260
                    gx = p2.tile([P, C, GXW], BF16, name="gx", tag="gx")
                    nc.vector.tensor_copy(out=gx[:, :, HD : HD + H], in_=exb[:])
                    exbc = exb[:].rearrange("p c (one h) -> p c one h", one=1)
                    nc.vector.tensor_tensor(
                        out=gx[:, :, 0 : DSPLIT * H].rearrange(
                            "p c (d h) -> p c d h", h=H
                        ),
                        in0=G[:, :, 16 : 16 + 2 * DSPLIT * H]
                        .bitcast(BF16)
                        .rearrange("p c (d h) -> p c d h", h=H),
                        in1=exbc.to_broadcast([P, C, DSPLIT, H]),
                        op=AL.mult,
                    )
                    ftail = p2s.tile([P, C, (D - DSPLIT) * H], BF16, name="ft", tag="ft")
                    nc.scalar.activation(
                        out=ftail[:],
                        in_=G[:, :, 16 + 2 * DSPLIT * H : ROWB].bitcast(F8),
                        func=ACT.Copy,
                    )
                    nc.vector.tensor_tensor(
                        out=gx[:, :, DSPLIT * H : HD].rearrange(
                            "p c (d h) -> p c d h", h=H
                        ),
                        in0=ftail[:].rearrange("p c (d h) -> p c d h", h=H),
                        in1=exbc.to_broadcast([P, C, D - DSPLIT, H]),
                        op=AL.mult,
                    )

                    out_ps = outps_pool.tile([P, GXW], F32, name="out_ps")
                    for j in range(C):
                        nc.tensor.matmul(
                            out_ps[:],
                            lhsT=st3_v[:, j, :],
                            rhs=gx[:, j, :],
                            start=(j == 0),
                            stop=(j == C - 1),
                        )

                    # epilogue: normalize, bias, tanh, mean over heads
                    rd0 = p2s.tile([P, H], F32, name="rd0", tag="rd0")
                    nc.vector.tensor_scalar(
                        out=rd0[:],
                        in0=out_ps[:, HD : HD + H],
                        scalar1=1e-9,
                        scalar2=None,
                        op0=AL.max,
                    )
                    rd = p2s.tile([P, H], F32, name="rd", tag="rd")
                    nc.vector.reciprocal(out=rd[:], in_=rd0[:])
                    nrm = p2.tile([P, HD], F32, name="nrm", tag="nrm")
                    nc.vector.tensor_tensor(
                        out=nrm[:].rearrange("p (d h) -> p d h", h=H),
                        in0=out_ps[:, 0:HD].rearrange("p (d h) -> p d h", h=H),
                        in1=rd[:]
                        .rearrange("p (one h) -> p one h", one=1)
                        .to_broadcast([P, D, H]),
                        op=AL.mult,
                    )
                    nb = p2.tile([P, HD], F32, name="nb", tag="nb")
                    nc.gpsimd.tensor_tensor(
                        out=nb[:], in0=nrm[:], in1=bias_sb[:], op=AL.add
                    )
                    th = p2.tile([P, HD], F32, name="th", tag="th")
                    nc.scalar.activation(out=th[:], in_=nb[:], func=ACT.Tanh)
                    thv = th[:].rearrange("p (d h) -> p d h", h=H)
                    m1 = p2s.tile([P, D], F32, name="m1", tag="m1")
                    nc.vector.tensor_tensor(
                        out=m1[:], in0=thv[:, :, 0], in1=thv[:, :, 1], op=AL.add
                    )
                    m2 = p2s.tile([P, D], F32, name="m2", tag="m2")
                    nc.gpsimd.tensor_tensor(
                        out=m2[:], in0=thv[:, :, 2], in1=thv[:, :, 3], op=AL.add
                    )
                    m3 = p2s.tile([P, D], F32, name="m3", tag="m3")
                    nc.vector.tensor_tensor(out=m3[:], in0=m1[:], in1=m2[:], op=AL.add)
                    nc.vector.tensor_scalar(
                        out=of_all[:, t, :],
                        in0=m3[:],
                        scalar1=0.25,
                        scalar2=None,
                        op0=AL.mult,
                    )
                    aux_off += W_t

                nc.sync.dma_start(
                    out=out[:, :].rearrange("(t p) d -> p t d", p=P), in_=of_all[:]
                )
    return nc


# --------------------------------------------------------------------------
# host entry
# --------------------------------------------------------------------------

def _make_static_inputs(W, attn_l, attn_r, bias):
    Wf = np.asarray(W, dtype=np.float32)
    ALRm = np.zeros((IN_DIM, 2 * H), dtype=np.float32)
    al = np.asarray(attn_l, dtype=np.float32)
    ar = np.asarray(attn_r, dtype=np.float32)
    for hh in range(H):
        ALRm[hh * D : (hh + 1) * D, hh] = al[hh]
        ALRm[hh * D : (hh + 1) * D, H + hh] = ar[hh]
    wcat = np.concatenate([Wf, np.ascontiguousarray(Wf.T), ALRm], axis=1)
    # bias in (d,h)-major layout
    b = np.asarray(bias, dtype=np.float32).reshape(H, D)
    bias_rep = np.tile(np.ascontiguousarray(b.T.reshape(1, HD)), (P, 1))
    return dict(
        WCAT=np.ascontiguousarray(wcat.astype(NP_BF16)),
        bias_dh=np.ascontiguousarray(bias_rep),
    )


def bench(nc, in_maps, n_iters=10):
    """Repeated-execution wall timing of the compiled SPMD kernel via PJRT."""
    import time

    import jax
    from jax.sharding import Mesh, NamedSharding, PartitionSpec
    from jax.experimental.shard_map import shard_map

    from concourse import bass2jax, mybir as _mb

    bass2jax.install_neuronx_cc_hook()
    n_cores = len(in_maps)
    in_names, out_names, out_avals, zero_outs = [], [], [], []
    partition_name = nc.partition_id_tensor.name if nc.partition_id_tensor else None
    for alloc in nc.m.functions[0].allocations:
        if not isinstance(alloc, _mb.MemoryLocationSet):
            continue
        name = alloc.memorylocations[0].name
        if alloc.kind == "ExternalInput":
            if name != partition_name:
                in_names.append(name)
        elif alloc.kind == "ExternalOutput":
            out_names.append(name)
            shape = tuple(alloc.tensor_shape)
            dtype = _mb.dt.np(alloc.dtype)
            out_avals.append(jax.core.ShapedArray(shape, dtype))
            zero_outs.append(np.zeros(shape, dtype))
    n_params = len(in_names)
    all_in_names = in_names + out_names
    if partition_name is not None:
        all_in_names.append(partition_name)

    def _body(*args):
        operands = list(args)
        if partition_name is not None:
            operands.append(bass2jax.partition_id_tensor())
        outs = bass2jax._bass_exec_p.bind(
            *operands,
            out_avals=tuple(out_avals),
            in_names=tuple(all_in_names),
            out_names=tuple(out_names),
            lowering_input_output_aliases=(),
            sim_require_finite=True,
            sim_require_nnan=True,
            nc=nc,
        )
        return tuple(outs)

    devices = jax.devices()[:n_cores]
    mesh = Mesh(np.asarray(devices), ("core",))
    n_outs = len(out_names)
    sharded = jax.jit(
        shard_map(
            _body,
            mesh=mesh,
            in_specs=(PartitionSpec("core"),) * (n_params + n_outs),
            out_specs=(PartitionSpec("core"),) * n_outs,
            check_rep=False,
        ),
        keep_unused=True,
    )
    sh = NamedSharding(mesh, PartitionSpec("core"))
    concat_in = [
        jax.device_put(
            np.concatenate([np.asarray(in_maps[c][nm]) for c in range(n_cores)], 0), sh
        )
        for nm in in_names
    ]
    concat_zeros = [
        jax.device_put(np.zeros((n_cores * z.shape[0], *z.shape[1:]), z.dtype), sh)
        for z in zero_outs
    ]
    outs = sharded(*concat_in, *concat_zeros)  # warmup/compile
    jax.block_until_ready(outs)
    times = []
    for _ in range(n_iters):
        t0 = time.perf_counter()
        outs = sharded(*concat_in, *concat_zeros)
        jax.block_until_ready(outs)
        times.append(time.perf_counter() - t0)
    results = [
        {
            nm: np.asarray(outs[i]).reshape(n_cores, *out_avals[i].shape)[c]
            for i, nm in enumerate(out_names)
        }
        for c in range(n_cores)
    ]
    return times, results


def kernel(h, W, attn_l, attn_r, bias, src, dst):
    from concourse.bass_utils import run_bass_kernel_spmd

    aux, clo, chi = preprocess(src, dst)
    static = _make_static_inputs(W, attn_l, attn_r, bias)
    nc = build_kernel(N_PAD, TILES, clo, chi)
    nc.compile()
    h_pad = np.zeros((N_PAD, IN_DIM), dtype=np.float32)
    h_pad[:N] = np.asarray(h, dtype=np.float32)
    in_maps = []
    for c in range(NC):
        m = dict(static)
        m["hT"] = np.ascontiguousarray(h_pad[aux[c]["perm"]].T).astype(NP_BF16)
        m["auxw"] = aux[c]["auxw"]
        in_maps.append(m)
    res = run_bass_kernel_spmd(nc, in_maps, core_ids=list(range(NC)), trace=False)
    out_full = np.zeros((N, D), dtype=np.float32)
    for c in range(NC):
        dev = res.results[c]["out"]  # [TILES*P, D]
        ids = aux[c]["member_ids"]  # [P, TILES]
        rows = ids.T.reshape(-1)  # row t*P+p  <->  ids[p, t]
        valid = rows < N
        out_full[rows[valid]] = dev[valid]
    kernel.last_nc = nc
    kernel.last_in_maps = in_maps
    kernel.last_aux = aux
    return out_full
